# revision 9
# baseline (speedup 1.0000x reference)
"""Trainium2 Bass kernel for nn_MultiHeadAttention (B=2, S=2048, D=1024, H=16, causal).

Sharding across 8 NeuronCores -- NO on-device collective:
  - Core c owns batch b=c//4 and head-group g=c%4 (4 heads).  Wq/Wk/Wv are
    column-sharded (256 features per core), Wo is row-sharded; each core
    emits a PARTIAL output projection over the full 2048 tokens and the
    host sums the 4 partials per batch at unshard time.  This removes the
    AllGather + barrier that cost ~190us in the token-sharded design.
  - Everything on-chip is bf16 (PSUM accumulation fp32); softmax scale is
    folded into Wk/bk on the host.  K/Q biases are per-partition
    tensor_scalar adds fused into the PSUM->SBUF copy; V bias is a rank-1
    matmul into PSUM.
  - Pipelined per 512-token chunk tc: project K/V/Q for chunk tc, run
    attention for query chunk tc (keys 0..512*tc+511), then project chunk
    tc+1 BEFORE the output projection of tc so the softmax-normalize tail
    never stalls the PE.
  - Attention per head pair (feature block = 128 partitions): score
    matmuls for the two heads are row-tiled (partitions 0:64 / 64:128)
    into one 2-bank PSUM tile so they run concurrently; one wide exp
    covers both heads; emission is software-pipelined (scores of block
    j+1 issue before ctx of block j) so the PE never waits on the scalar
    engine's exp.  Diagonal key blocks are width-trimmed to 512-128*o
    columns; the causal mask restricted to the computed window is the
    same [128,2,128] staircase for every block (beyond 128 columns it is
    all-ones).  ctx accumulates in PSUM across key blocks with a 65th
    stationary V column (= softmax denominator); ctx is copied raw to
    SBUF right away to free the PSUM banks for the next pair, and the
    normalize chain (recip = exp(-ln(d)) on scalar, rank-1 replicate,
    DVE mult) runs off the critical path.
"""
import numpy as np
import ml_dtypes

import concourse.bass as bass
import concourse.bacc as bacc
import concourse.mybir as mybir
import concourse.tile as tile
from concourse.bass_utils import run_bass_kernel_spmd

B, S, D, H, HD = 2, 2048, 1024, 16, 64
NC = 8
P = 128
F32 = mybir.dt.float32
BF = mybir.dt.bfloat16
NPBF = ml_dtypes.bfloat16

TRACE = False        # set True (e.g. from test.py) to capture an NTFF profile
LAST_RESULT = None   # BassKernelResults of the most recent kernel() call

_ACT_PATCHED = False


def _patch_act_tables():
    """Steer Bacc's act-table-load pass to the combined natural_log+exp
    set so a kernel using both Exp and Ln takes ONE table load instead of
    alternating between table sets (~1.3us per switch on scalar)."""
    global _ACT_PATCHED
    if _ACT_PATCHED:
        return
    import concourse.bacc as _bacc
    _orig = _bacc.get_activation_tables

    def _filtered(arch):
        t = _orig(arch)
        fexp = mybir.ActivationFunctionType.Exp
        fln = mybir.ActivationFunctionType.Ln
        out = {}
        for name, fns in t.items():
            if name != "natural_log_exp_and_others" and (
                    fexp in fns or fln in fns):
                fns = fns - {fexp, fln}
            out[name] = fns
        return out

    _bacc.get_activation_tables = _filtered
    _ACT_PATCHED = True


def _emit(causal: bool):
    nc = bacc.Bacc(trn_type="TRN2", num_devices=NC)
    fexp = mybir.ActivationFunctionType.Exp
    fln = mybir.ActivationFunctionType.Ln
    _patch_act_tables()

    xT = nc.dram_tensor("xT", [D, S], BF, kind="ExternalInput")
    wqT = nc.dram_tensor("wqT", [D, 256], BF, kind="ExternalInput")
    wkT = nc.dram_tensor("wkT", [D, 256], BF, kind="ExternalInput")
    wvT = nc.dram_tensor("wvT", [D, 256], BF, kind="ExternalInput")
    woT = nc.dram_tensor("woT", [256, D], BF, kind="ExternalInput")
    bqc_d = nc.dram_tensor("bqc", [P, 2], F32, kind="ExternalInput")
    bkc_d = nc.dram_tensor("bkc", [P, 2], F32, kind="ExternalInput")
    bv_d = nc.dram_tensor("bv", [1, 256], BF, kind="ExternalInput")
    if causal:
        cm_d = nc.dram_tensor("cm", [P, 2, P], BF, kind="ExternalInput")
    outT = nc.dram_tensor("outT", [D, S], BF, kind="ExternalOutput")

    with tile.TileContext(nc) as tc, \
         tc.tile_pool(name="const", bufs=1) as const, \
         tc.tile_pool(name="big", bufs=1) as big, \
         tc.tile_pool(name="oio", bufs=3) as oio, \
         tc.tile_pool(name="ex", bufs=4) as ex, \
         tc.tile_pool(name="u", bufs=4) as up, \
         tc.tile_pool(name="sm", bufs=2) as sm, \
         tc.tile_pool(name="ps_a", bufs=2, space="PSUM") as ps_a, \
         tc.tile_pool(name="ps_b", bufs=4, space="PSUM") as ps_b:

        # ---------- constants / inputs ----------
        ones = const.tile([1, P], BF)
        nc.gpsimd.memset(ones[:], 1.0)
        bqc_sb = const.tile([P, 2], F32)
        nc.scalar.dma_start(bqc_sb[:], bqc_d[:])
        bkc_sb = const.tile([P, 2], F32)
        nc.scalar.dma_start(bkc_sb[:], bkc_d[:])
        bv_sb = const.tile([1, 256], BF)
        nc.scalar.dma_start(bv_sb[:], bv_d[:])
        if causal:
            cm_sb = const.tile([P, 2, P], BF)
            nc.scalar.dma_start(cm_sb[:], cm_d[:])

        wk_sb = big.tile([P, 8, 256], BF)
        wv_sb = big.tile([P, 8, 256], BF)
        wq_sb = big.tile([P, 8, 256], BF)
        wo_sb = big.tile([P, 2, D], BF)
        xt_sb = big.tile([P, 8, S], BF)
        kt_sb = big.tile([P, 2, S], BF)
        qt_sb = big.tile([P, 2, S], BF)
        v_sb = big.tile([P, 16, 4, 65], BF)
        ctx_sb = big.tile([P, 2, S], BF)
        nc.gpsimd.memset(v_sb[:, :, :, 64:65], 1.0)

        wkr = wkT.rearrange("(o p) f -> p o f", p=P)
        wvr = wvT.rearrange("(o p) f -> p o f", p=P)
        wqr = wqT.rearrange("(o p) f -> p o f", p=P)
        wor = woT.rearrange("(o p) f -> p o f", p=P)
        xr = xT.rearrange("(o p) t -> p o t", p=P)
        outr = outT.rearrange("(o p) t -> p o t", p=P)

        # Input DMAs are spread across issue queues so the transfers
        # overlap: sync gets wk + x chunk 0 (feeds the first K matmuls),
        # scalar gets wv/wq, gpsimd gets the rest of x and wo.
        nc.sync.dma_start(wk_sb[:], wkr[:])
        for kt in range(8):
            nc.sync.dma_start(xt_sb[:, kt, 0:512], xr[:, kt, 0:512])
        nc.scalar.dma_start(wv_sb[:], wvr[:])
        nc.scalar.dma_start(wq_sb[:], wqr[:])
        for kt in range(8):
            nc.gpsimd.dma_start(xt_sb[:, kt, 512:2048], xr[:, kt, 512:2048])
        nc.gpsimd.dma_start(wo_sb[:], wor[:])

        def proj_chunk(tc_i):
            t0 = 512 * tc_i
            # K^T and Q^T: out[feat, tok], feature block fb == head pair;
            # bias is a per-partition scalar add fused into the copy.
            for w_sb, b_sb, sink in ((wk_sb, bkc_sb, kt_sb),
                                     (wq_sb, bqc_sb, qt_sb)):
                for fb in range(2):
                    pt = ps_b.tile([P, 512], F32, tag="psb")
                    for kt in range(8):
                        nc.tensor.matmul(
                            pt[:], w_sb[:, kt, 128 * fb:128 * fb + 128],
                            xt_sb[:, kt, t0:t0 + 512],
                            start=(kt == 0), stop=(kt == 7))
                    nc.vector.tensor_scalar_add(
                        sink[:, fb, t0:t0 + 512], pt[:], b_sb[:, fb:fb + 1])
            # V: out[tok, feat] per 128-token block (65th col pre-set to 1)
            for tb in range(4):
                jb = 4 * tc_i + tb
                pt = ps_b.tile([P, 512], F32, tag="psb")
                for kt in range(8):
                    nc.tensor.matmul(
                        pt[:, 0:256],
                        xt_sb[:, kt, t0 + 128 * tb:t0 + 128 * tb + 128],
                        wv_sb[:, kt, :], start=(kt == 0), stop=False)
                nc.tensor.matmul(
                    pt[:, 0:256], ones[0:1, 0:P], bv_sb[0:1, :],
                    start=False, stop=True)
                nc.vector.tensor_copy(
                    v_sb[:, jb, :, 0:64],
                    pt[:, 0:256].rearrange("p (h d) -> p h d", h=4))

        def attn_chunk(tc_i):
            """Scores+exp+ctx for both head pairs, interleaved so the PE
            always has the other pair's work between a block's scores and
            its exp-dependent ctx matmuls.  Returns the raw [65,512] SBUF
            ctx copies (4 heads) for deferred normalization."""
            t0 = 512 * tc_i
            jn = 4 * tc_i + 4 if causal else 16
            ctx = [[None, None], [None, None]]
            for _pr in range(2):
                for _hh in range(2):
                    ctile = ps_b.tile([P, 512], F32, tag="psb")
                    ctx[_pr][_hh] = ctile
            prev = [None, None]

            def emit_ctx(pair, pj, pet, pqo, pwid):
                for hh in range(2):
                    nc.tensor.matmul(
                        ctx[pair][hh][0:65, pqo:pqo + pwid],
                        v_sb[:, pj, 2 * pair + hh, :], pet[:, hh, 0:pwid],
                        start=(pj == 0), stop=(pj == jn - 1))

            for j in range(jn):
                o_ = j - 4 * tc_i if causal else -1
                qo = 0 if o_ < 0 else 128 * o_
                wid = 512 - qo
                for pair in range(2):
                    sc = ps_a.tile([P, 1024], F32, tag="psa")
                    for hh in range(2):
                        nc.tensor.matmul(
                            sc[:, 512 * hh:512 * hh + wid],
                            kt_sb[64 * hh:64 * hh + 64, pair,
                                  128 * j:128 * j + 128],
                            qt_sb[64 * hh:64 * hh + 64, pair,
                                  t0 + qo:t0 + qo + wid],
                            start=True, stop=True)
                    et = ex.tile([P, 2, 512], BF, tag="exp")
                    if wid == 512:
                        nc.scalar.activation(et[:, :, :], sc[:, :], fexp)
                    else:
                        nc.scalar.activation(
                            et[:, :, 0:wid],
                            sc[:].rearrange("p (s n) -> p s n", s=2)
                            [:, :, 0:wid], fexp)
                    if o_ >= 0:
                        nc.vector.tensor_tensor(
                            et[:, :, 0:P], et[:, :, 0:P], cm_sb[:],
                            mybir.AluOpType.mult)
                    if prev[pair] is not None:
                        emit_ctx(pair, *prev[pair])
                    prev[pair] = (j, et, qo, wid)
            us = []
            for pair in range(2):
                emit_ctx(pair, *prev[pair])
                for hh in range(2):
                    u = up.tile([65, 512], F32, tag="u")
                    nc.vector.tensor_copy(u[:], ctx[pair][hh][0:65, :])
                    us.append(u)
            return us

        def norm_chunk(tc_i, us):
            """recip = exp(-ln(denominator)); even head lands on partitions
            0:64 of ctx_sb, odd heads are collected in a [64,2,512] tile
            and moved to partitions 64:128 with ONE SBUF->SBUF DMA
            (issued from the idle gpsimd queue)."""
            t0 = 512 * tc_i
            ctmp = sm.tile([64, 2, 512], BF, tag="ctmp")
            for pair in range(2):
                for hh in range(2):
                    u = us[2 * pair + hh]
                    lnd = sm.tile([1, 512], F32, tag="lnd")
                    nc.scalar.activation(lnd[:], u[64:65, 0:512], fln)
                    rcp = sm.tile([1, 512], BF, tag="rcp")
                    nc.scalar.activation(rcp[:], lnd[:], fexp, scale=-1.0)
                    rep_ps = ps_a.tile([P, 1024], F32, tag="psa")
                    nc.tensor.matmul(rep_ps[0:64, 0:512], ones[0:1, 0:64],
                                     rcp[0:1, :], start=True, stop=True)
                    rep = sm.tile([64, 512], F32, tag="rep")
                    nc.vector.tensor_copy(rep[:], rep_ps[0:64, 0:512])
                    if hh == 0:
                        nc.vector.tensor_tensor(
                            ctx_sb[0:64, pair, t0:t0 + 512], u[0:64, :],
                            rep[:], mybir.AluOpType.mult)
                    else:
                        nc.vector.tensor_tensor(
                            ctmp[:, pair, :], u[0:64, :], rep[:],
                            mybir.AluOpType.mult)
            nc.gpsimd.dma_start(ctx_sb[64:128, 0:2, t0:t0 + 512], ctmp[:])

        def outproj_chunk(tc_i):
            t0 = 512 * tc_i
            for m in range(8):
                pt = ps_b.tile([P, 512], F32, tag="psb")
                for kt in range(2):
                    nc.tensor.matmul(
                        pt[:], wo_sb[:, kt, 128 * m:128 * m + 128],
                        ctx_sb[:, kt, t0:t0 + 512],
                        start=(kt == 0), stop=(kt == 1))
                t = oio.tile([P, 512], BF, tag="oio")
                nc.vector.tensor_copy(t[:], pt[:])
                nc.sync.dma_start(outr[:, m, t0:t0 + 512], t[:])

        if causal:
            proj_chunk(0)
            for tc_i in range(4):
                us = attn_chunk(tc_i)
                if tc_i < 3:
                    proj_chunk(tc_i + 1)
                norm_chunk(tc_i, us)
                outproj_chunk(tc_i)
        else:
            for tc_i in range(4):
                proj_chunk(tc_i)
            for tc_i in range(4):
                us = attn_chunk(tc_i)
                norm_chunk(tc_i, us)
                outproj_chunk(tc_i)

    nc.compile()
    return nc


_CACHE = {}


def _get_nc(causal: bool):
    if causal not in _CACHE:
        _CACHE[causal] = _emit(causal)
    return _CACHE[causal]


def kernel(**inputs):
    x = np.asarray(inputs["x"], dtype=np.float32)
    Wq = np.asarray(inputs["Wq"], dtype=np.float32)
    bq = np.asarray(inputs["bq"], dtype=np.float32)
    Wk = np.asarray(inputs["Wk"], dtype=np.float32)
    bk = np.asarray(inputs["bk"], dtype=np.float32)
    Wv = np.asarray(inputs["Wv"], dtype=np.float32)
    bv = np.asarray(inputs["bv"], dtype=np.float32)
    Wo = np.asarray(inputs["Wo"], dtype=np.float32)
    bo = np.asarray(inputs["bo"], dtype=np.float32)
    causal = bool(int(np.asarray(inputs["enable_causal"])))

    scale = np.float32(1.0 / np.sqrt(HD))
    xTb = [np.ascontiguousarray(x[b].T).astype(NPBF) for b in range(B)]
    cm = np.ascontiguousarray(np.broadcast_to(
        (np.arange(P)[:, None] <= np.arange(P)[None, :])
        .astype(np.float32)[:, None, :], (P, 2, P))).astype(NPBF)

    nc = _get_nc(causal)
    in_maps = []
    for c in range(NC):
        b, g = divmod(c, 4)
        f0 = 256 * g
        m = {"xT": xTb[b],
             "wqT": np.ascontiguousarray(Wq[f0:f0 + 256, :].T).astype(NPBF),
             "wkT": np.ascontiguousarray(
                 (Wk[f0:f0 + 256, :] * scale).T).astype(NPBF),
             "wvT": np.ascontiguousarray(Wv[f0:f0 + 256, :].T).astype(NPBF),
             "woT": np.ascontiguousarray(Wo[:, f0:f0 + 256].T).astype(NPBF),
             "bqc": np.ascontiguousarray(
                 bq[f0:f0 + 256].reshape(2, P).T).astype(np.float32),
             "bkc": np.ascontiguousarray(
                 (bk[f0:f0 + 256] * scale).reshape(2, P).T).astype(np.float32),
             "bv": bv[f0:f0 + 256].reshape(1, 256).astype(NPBF)}
        if causal:
            m["cm"] = cm
        in_maps.append(m)

    global LAST_RESULT
    res = run_bass_kernel_spmd(nc, in_maps, list(range(NC)), trace=TRACE)
    LAST_RESULT = res
    out = np.zeros((B, S, D), dtype=np.float32)
    for c in range(NC):
        b = c // 4
        out[b] += np.asarray(res.results[c]["outT"], dtype=np.float32).T
    out += bo[None, None, :]
    return out


# revision 10
# speedup vs baseline: 1.1215x; 1.1215x over previous
"""Trainium2 Bass kernel for nn_MultiHeadAttention (B=2, S=2048, D=1024, H=16, causal).

Sharding across 8 NeuronCores -- NO on-device collective:
  - Core c owns batch b=c//4 and head-group g=c%4 (4 heads).  Wq/Wk/Wv are
    column-sharded (256 features per core), Wo is row-sharded; each core
    emits a PARTIAL output projection over the full 2048 tokens and the
    host sums the 4 partials per batch at unshard time.  This removes the
    AllGather + barrier that cost ~190us in the token-sharded design.
  - Everything on-chip is bf16 (PSUM accumulation fp32); softmax scale is
    folded into Wk/bk on the host.  K/Q biases are per-partition
    tensor_scalar adds fused into the PSUM->SBUF copy; V bias is a rank-1
    matmul into PSUM.
  - Pipelined per 512-token chunk tc: project K/V/Q for chunk tc, run
    attention for query chunk tc (keys 0..512*tc+511), then project chunk
    tc+1 BEFORE the output projection of tc so the softmax-normalize tail
    never stalls the PE.
  - Attention per head pair (feature block = 128 partitions): score
    matmuls for the two heads are row-tiled (partitions 0:64 / 64:128)
    into one 2-bank PSUM tile so they run concurrently; one wide exp
    covers both heads; emission is software-pipelined (scores of block
    j+1 issue before ctx of block j) so the PE never waits on the scalar
    engine's exp.  Diagonal key blocks are width-trimmed to 512-128*o
    columns; the causal mask restricted to the computed window is the
    same [128,2,128] staircase for every block (beyond 128 columns it is
    all-ones).  ctx accumulates in PSUM across key blocks with a 65th
    stationary V column (= softmax denominator); ctx is copied raw to
    SBUF right away to free the PSUM banks for the next pair, and the
    normalize chain (recip = exp(-ln(d)) on scalar, rank-1 replicate,
    DVE mult) runs off the critical path.
"""
import numpy as np
import ml_dtypes

import concourse.bass as bass
import concourse.bacc as bacc
import concourse.mybir as mybir
import concourse.tile as tile
from concourse.bass_utils import run_bass_kernel_spmd

B, S, D, H, HD = 2, 2048, 1024, 16, 64
NC = 8
P = 128
F32 = mybir.dt.float32
BF = mybir.dt.bfloat16
NPBF = ml_dtypes.bfloat16

TRACE = False        # set True (e.g. from test.py) to capture an NTFF profile
LAST_RESULT = None   # BassKernelResults of the most recent kernel() call

_ACT_PATCHED = False


def _patch_act_tables():
    """Steer Bacc's act-table-load pass to the combined natural_log+exp
    set so a kernel using both Exp and Ln takes ONE table load instead of
    alternating between table sets (~1.3us per switch on scalar)."""
    global _ACT_PATCHED
    if _ACT_PATCHED:
        return
    import concourse.bacc as _bacc
    _orig = _bacc.get_activation_tables

    def _filtered(arch):
        t = _orig(arch)
        fexp = mybir.ActivationFunctionType.Exp
        fln = mybir.ActivationFunctionType.Ln
        out = {}
        for name, fns in t.items():
            if name != "natural_log_exp_and_others" and (
                    fexp in fns or fln in fns):
                fns = fns - {fexp, fln}
            out[name] = fns
        return out

    _bacc.get_activation_tables = _filtered
    _ACT_PATCHED = True


def _emit(causal: bool):
    nc = bacc.Bacc(trn_type="TRN2", num_devices=NC)
    fexp = mybir.ActivationFunctionType.Exp
    fln = mybir.ActivationFunctionType.Ln
    _patch_act_tables()

    xT = nc.dram_tensor("xT", [D, S], BF, kind="ExternalInput")
    wqT = nc.dram_tensor("wqT", [D, 256], BF, kind="ExternalInput")
    wkT = nc.dram_tensor("wkT", [D, 256], BF, kind="ExternalInput")
    wvT = nc.dram_tensor("wvT", [D, 256], BF, kind="ExternalInput")
    woT = nc.dram_tensor("woT", [256, D], BF, kind="ExternalInput")
    bqc_d = nc.dram_tensor("bqc", [P, 2], F32, kind="ExternalInput")
    bkc_d = nc.dram_tensor("bkc", [P, 2], F32, kind="ExternalInput")
    bv_d = nc.dram_tensor("bv", [1, 256], BF, kind="ExternalInput")
    if causal:
        cm_d = nc.dram_tensor("cm", [P, 2, P], BF, kind="ExternalInput")
    outT = nc.dram_tensor("outT", [D, S], BF, kind="ExternalOutput")

    with tile.TileContext(nc) as tc, \
         tc.tile_pool(name="const", bufs=1) as const, \
         tc.tile_pool(name="big", bufs=1) as big, \
         tc.tile_pool(name="oio", bufs=3) as oio, \
         tc.tile_pool(name="ex", bufs=4) as ex, \
         tc.tile_pool(name="u", bufs=4) as up, \
         tc.tile_pool(name="sm", bufs=2) as sm, \
         tc.tile_pool(name="ps_a", bufs=2, space="PSUM") as ps_a, \
         tc.tile_pool(name="ps_c", bufs=2, space="PSUM") as ps_c, \
         tc.tile_pool(name="ps_w", bufs=2, space="PSUM") as ps_w:

        # ---------- constants / inputs ----------
        ones = const.tile([1, P], BF)
        nc.gpsimd.memset(ones[:], 1.0)
        bqc_sb = const.tile([P, 2], F32)
        nc.scalar.dma_start(bqc_sb[:], bqc_d[:])
        bkc_sb = const.tile([P, 2], F32)
        nc.scalar.dma_start(bkc_sb[:], bkc_d[:])
        bv_sb = const.tile([1, 256], BF)
        nc.scalar.dma_start(bv_sb[:], bv_d[:])
        if causal:
            cm_sb = const.tile([P, 2, P], BF)
            nc.scalar.dma_start(cm_sb[:], cm_d[:])

        wk_sb = big.tile([P, 8, 256], BF)
        wv_sb = big.tile([P, 8, 256], BF)
        wq_sb = big.tile([P, 8, 256], BF)
        wo_sb = big.tile([P, 2, D], BF)
        xt_sb = big.tile([P, 8, S], BF)
        kt_sb = big.tile([P, 2, S], BF)
        qt_sb = big.tile([P, 2, S], BF)
        v_sb = big.tile([P, 16, 4, 65], BF)
        ctx_sb = big.tile([P, 2, S], BF)
        nc.gpsimd.memset(v_sb[:, :, :, 64:65], 1.0)

        wkr = wkT.rearrange("(o p) f -> p o f", p=P)
        wvr = wvT.rearrange("(o p) f -> p o f", p=P)
        wqr = wqT.rearrange("(o p) f -> p o f", p=P)
        wor = woT.rearrange("(o p) f -> p o f", p=P)
        xr = xT.rearrange("(o p) t -> p o t", p=P)
        outr = outT.rearrange("(o p) t -> p o t", p=P)

        # Input DMAs: the critical path is wk + x chunk 0 (feeds the
        # first K matmuls).  gpsimd issues x chunk 0 first, THEN the bulk
        # of x, so the bulk never competes with the startup-critical
        # bytes; sync carries the weights; scalar the small constants.
        nc.sync.dma_start(wk_sb[:], wkr[:])
        nc.sync.dma_start(wv_sb[:], wvr[:])
        nc.sync.dma_start(wq_sb[:], wqr[:])
        for kt in range(8):
            nc.gpsimd.dma_start(xt_sb[:, kt, 0:512], xr[:, kt, 0:512])
        for kt in range(8):
            nc.gpsimd.dma_start(xt_sb[:, kt, 512:2048], xr[:, kt, 512:2048])
        nc.sync.dma_start(wo_sb[:], wor[:])

        def proj_chunk(tc_i):
            t0 = 512 * tc_i
            # K^T and Q^T: out[feat, tok], feature block fb == head pair;
            # bias is a per-partition scalar add fused into the copy.
            for w_sb, b_sb, sink in ((wk_sb, bkc_sb, kt_sb),
                                     (wq_sb, bqc_sb, qt_sb)):
                for fb in range(2):
                    pt = ps_w.tile([P, 512], F32, tag="psw")
                    for kt in range(8):
                        nc.tensor.matmul(
                            pt[:], w_sb[:, kt, 128 * fb:128 * fb + 128],
                            xt_sb[:, kt, t0:t0 + 512],
                            start=(kt == 0), stop=(kt == 7))
                    nc.vector.tensor_scalar_add(
                        sink[:, fb, t0:t0 + 512], pt[:], b_sb[:, fb:fb + 1])
            # V: out[tok, feat] per 128-token block (65th col pre-set to 1)
            for tb in range(4):
                jb = 4 * tc_i + tb
                pt = ps_w.tile([P, 512], F32, tag="psw")
                for kt in range(8):
                    nc.tensor.matmul(
                        pt[:, 0:256],
                        xt_sb[:, kt, t0 + 128 * tb:t0 + 128 * tb + 128],
                        wv_sb[:, kt, :], start=(kt == 0), stop=False)
                nc.tensor.matmul(
                    pt[:, 0:256], ones[0:1, 0:P], bv_sb[0:1, :],
                    start=False, stop=True)
                nc.vector.tensor_copy(
                    v_sb[:, jb, :, 0:64],
                    pt[:, 0:256].rearrange("p (h d) -> p h d", h=4))

        def attn_chunk(tc_i):
            """Scores+exp+ctx for both head pairs.  One flat software
            pipeline over blocks (pair-major): the ctx matmuls of block b
            issue after the scores of block b+1, across the pair boundary
            too, so the PE never waits on the scalar engine's exp."""
            t0 = 512 * tc_i
            jn = 4 * tc_i + 4 if causal else 16
            ctx = {}
            prev = None
            us = []

            def emit_ctx(pair, pj, pet, pqo, pwid):
                for hh in range(2):
                    nc.tensor.matmul(
                        ctx[pair][hh][0:65, pqo:pqo + pwid],
                        v_sb[:, pj, 2 * pair + hh, :], pet[:, hh, 0:pwid],
                        start=(pj == 0), stop=(pj == jn - 1))
                if pj == jn - 1:
                    for hh in range(2):
                        u = up.tile([65, 512], F32, tag="u")
                        nc.vector.tensor_copy(u[:], ctx[pair][hh][0:65, :])
                        us.append(u)

            for pair in range(2):
                c0 = ps_c.tile([P, 512], F32, tag="ctx")
                c1 = ps_c.tile([P, 512], F32, tag="ctx")
                ctx[pair] = (c0, c1)
                for j in range(jn):
                    o_ = j - 4 * tc_i if causal else -1
                    qo = 0 if o_ < 0 else 128 * o_
                    wid = 512 - qo
                    sc = ps_a.tile([P, 1024], F32, tag="psa")
                    for hh in range(2):
                        nc.tensor.matmul(
                            sc[:, 512 * hh:512 * hh + wid],
                            kt_sb[64 * hh:64 * hh + 64, pair,
                                  128 * j:128 * j + 128],
                            qt_sb[64 * hh:64 * hh + 64, pair,
                                  t0 + qo:t0 + qo + wid],
                            start=True, stop=True)
                    et = ex.tile([P, 2, 512], BF, tag="exp")
                    if wid == 512:
                        nc.scalar.activation(et[:, :, :], sc[:, :], fexp)
                    else:
                        nc.scalar.activation(
                            et[:, :, 0:wid],
                            sc[:].rearrange("p (s n) -> p s n", s=2)
                            [:, :, 0:wid], fexp)
                    if o_ >= 0:
                        nc.vector.tensor_tensor(
                            et[:, :, 0:P], et[:, :, 0:P], cm_sb[:],
                            mybir.AluOpType.mult)
                    if prev is not None:
                        emit_ctx(*prev)
                    prev = (pair, j, et, qo, wid)
            emit_ctx(*prev)
            return us

        def norm_chunk(tc_i, us):
            """recip = exp(-ln(denominator)); even head lands on partitions
            0:64 of ctx_sb, odd heads are collected in a [64,2,512] tile
            and moved to partitions 64:128 with ONE SBUF->SBUF DMA
            (issued from the idle gpsimd queue)."""
            t0 = 512 * tc_i
            ctmp = sm.tile([64, 2, 512], BF, tag="ctmp")
            for pair in range(2):
                for hh in range(2):
                    u = us[2 * pair + hh]
                    lnd = sm.tile([1, 512], F32, tag="lnd")
                    nc.scalar.activation(lnd[:], u[64:65, 0:512], fln)
                    rcp = sm.tile([1, 512], BF, tag="rcp")
                    nc.scalar.activation(rcp[:], lnd[:], fexp, scale=-1.0)
                    rep_ps = ps_w.tile([P, 512], F32, tag="psw")
                    nc.tensor.matmul(rep_ps[0:64, :], ones[0:1, 0:64],
                                     rcp[0:1, :], start=True, stop=True)
                    rep = sm.tile([64, 512], F32, tag="rep")
                    nc.vector.tensor_copy(rep[:], rep_ps[0:64, :])
                    if hh == 0:
                        nc.vector.tensor_tensor(
                            ctx_sb[0:64, pair, t0:t0 + 512], u[0:64, :],
                            rep[:], mybir.AluOpType.mult)
                    else:
                        nc.vector.tensor_tensor(
                            ctmp[:, pair, :], u[0:64, :], rep[:],
                            mybir.AluOpType.mult)
            nc.gpsimd.dma_start(ctx_sb[64:128, 0:2, t0:t0 + 512], ctmp[:])

        def outproj_chunk(tc_i):
            t0 = 512 * tc_i
            for m in range(8):
                pt = ps_w.tile([P, 512], F32, tag="psw")
                for kt in range(2):
                    nc.tensor.matmul(
                        pt[:], wo_sb[:, kt, 128 * m:128 * m + 128],
                        ctx_sb[:, kt, t0:t0 + 512],
                        start=(kt == 0), stop=(kt == 1))
                t = oio.tile([P, 512], BF, tag="oio")
                nc.vector.tensor_copy(t[:], pt[:])
                nc.sync.dma_start(outr[:, m, t0:t0 + 512], t[:])

        if causal:
            proj_chunk(0)
            for tc_i in range(4):
                us = attn_chunk(tc_i)
                if tc_i < 3:
                    proj_chunk(tc_i + 1)
                norm_chunk(tc_i, us)
                outproj_chunk(tc_i)
        else:
            for tc_i in range(4):
                proj_chunk(tc_i)
            for tc_i in range(4):
                us = attn_chunk(tc_i)
                norm_chunk(tc_i, us)
                outproj_chunk(tc_i)

    nc.compile()
    return nc


_CACHE = {}


def _get_nc(causal: bool):
    if causal not in _CACHE:
        _CACHE[causal] = _emit(causal)
    return _CACHE[causal]


def kernel(**inputs):
    x = np.asarray(inputs["x"], dtype=np.float32)
    Wq = np.asarray(inputs["Wq"], dtype=np.float32)
    bq = np.asarray(inputs["bq"], dtype=np.float32)
    Wk = np.asarray(inputs["Wk"], dtype=np.float32)
    bk = np.asarray(inputs["bk"], dtype=np.float32)
    Wv = np.asarray(inputs["Wv"], dtype=np.float32)
    bv = np.asarray(inputs["bv"], dtype=np.float32)
    Wo = np.asarray(inputs["Wo"], dtype=np.float32)
    bo = np.asarray(inputs["bo"], dtype=np.float32)
    causal = bool(int(np.asarray(inputs["enable_causal"])))

    scale = np.float32(1.0 / np.sqrt(HD))
    xTb = [np.ascontiguousarray(x[b].T).astype(NPBF) for b in range(B)]
    cm = np.ascontiguousarray(np.broadcast_to(
        (np.arange(P)[:, None] <= np.arange(P)[None, :])
        .astype(np.float32)[:, None, :], (P, 2, P))).astype(NPBF)

    nc = _get_nc(causal)
    in_maps = []
    for c in range(NC):
        b, g = divmod(c, 4)
        f0 = 256 * g
        m = {"xT": xTb[b],
             "wqT": np.ascontiguousarray(Wq[f0:f0 + 256, :].T).astype(NPBF),
             "wkT": np.ascontiguousarray(
                 (Wk[f0:f0 + 256, :] * scale).T).astype(NPBF),
             "wvT": np.ascontiguousarray(Wv[f0:f0 + 256, :].T).astype(NPBF),
             "woT": np.ascontiguousarray(Wo[:, f0:f0 + 256].T).astype(NPBF),
             "bqc": np.ascontiguousarray(
                 bq[f0:f0 + 256].reshape(2, P).T).astype(np.float32),
             "bkc": np.ascontiguousarray(
                 (bk[f0:f0 + 256] * scale).reshape(2, P).T).astype(np.float32),
             "bv": bv[f0:f0 + 256].reshape(1, 256).astype(NPBF)}
        if causal:
            m["cm"] = cm
        in_maps.append(m)

    global LAST_RESULT
    res = run_bass_kernel_spmd(nc, in_maps, list(range(NC)), trace=TRACE)
    LAST_RESULT = res
    out = np.zeros((B, S, D), dtype=np.float32)
    for c in range(NC):
        b = c // 4
        out[b] += np.asarray(res.results[c]["outT"], dtype=np.float32).T
    out += bo[None, None, :]
    return out


# revision 11
# speedup vs baseline: 1.1255x; 1.0035x over previous
"""Trainium2 Bass kernel for nn_MultiHeadAttention (B=2, S=2048, D=1024, H=16, causal).

Sharding across 8 NeuronCores -- NO on-device collective:
  - Core c owns batch b=c//4 and head-group g=c%4 (4 heads).  Wq/Wk/Wv are
    column-sharded (256 features per core), Wo is row-sharded; each core
    emits a PARTIAL output projection over the full 2048 tokens and the
    host sums the 4 partials per batch at unshard time.  This removes the
    AllGather + barrier that cost ~190us in the token-sharded design.
  - Everything on-chip is bf16 (PSUM accumulation fp32); softmax scale is
    folded into Wk/bk on the host.  K/Q biases are per-partition
    tensor_scalar adds fused into the PSUM->SBUF copy; V bias is a rank-1
    matmul into PSUM.
  - Pipelined per 512-token chunk tc: project K/V/Q for chunk tc, run
    attention for query chunk tc (keys 0..512*tc+511), then project chunk
    tc+1 BEFORE the output projection of tc so the softmax-normalize tail
    never stalls the PE.
  - Attention per head pair (feature block = 128 partitions): score
    matmuls for the two heads are row-tiled (partitions 0:64 / 64:128)
    into one 2-bank PSUM tile so they run concurrently; one wide exp
    covers both heads; emission is software-pipelined (scores of block
    j+1 issue before ctx of block j) so the PE never waits on the scalar
    engine's exp.  Diagonal key blocks are width-trimmed to 512-128*o
    columns; the causal mask restricted to the computed window is the
    same [128,2,128] staircase for every block (beyond 128 columns it is
    all-ones).  ctx accumulates in PSUM across key blocks with a 65th
    stationary V column (= softmax denominator); ctx is copied raw to
    SBUF right away to free the PSUM banks for the next pair, and the
    normalize chain (recip = exp(-ln(d)) on scalar, rank-1 replicate,
    DVE mult) runs off the critical path.
"""
import numpy as np
import ml_dtypes

import concourse.bass as bass
import concourse.bacc as bacc
import concourse.mybir as mybir
import concourse.tile as tile
from concourse.bass_utils import run_bass_kernel_spmd

B, S, D, H, HD = 2, 2048, 1024, 16, 64
NC = 8
P = 128
F32 = mybir.dt.float32
BF = mybir.dt.bfloat16
NPBF = ml_dtypes.bfloat16

TRACE = False        # set True (e.g. from test.py) to capture an NTFF profile
LAST_RESULT = None   # BassKernelResults of the most recent kernel() call

_ACT_PATCHED = False


def _patch_act_tables():
    """Steer Bacc's act-table-load pass to the combined natural_log+exp
    set so a kernel using both Exp and Ln takes ONE table load instead of
    alternating between table sets (~1.3us per switch on scalar)."""
    global _ACT_PATCHED
    if _ACT_PATCHED:
        return
    import concourse.bacc as _bacc
    _orig = _bacc.get_activation_tables

    def _filtered(arch):
        t = _orig(arch)
        fexp = mybir.ActivationFunctionType.Exp
        fln = mybir.ActivationFunctionType.Ln
        out = {}
        for name, fns in t.items():
            if name != "natural_log_exp_and_others" and (
                    fexp in fns or fln in fns):
                fns = fns - {fexp, fln}
            out[name] = fns
        return out

    _bacc.get_activation_tables = _filtered
    _ACT_PATCHED = True


def _emit(causal: bool):
    nc = bacc.Bacc(trn_type="TRN2", num_devices=NC)
    fexp = mybir.ActivationFunctionType.Exp
    fln = mybir.ActivationFunctionType.Ln
    _patch_act_tables()

    xT = nc.dram_tensor("xT", [D, S], BF, kind="ExternalInput")
    wqT = nc.dram_tensor("wqT", [D, 256], BF, kind="ExternalInput")
    wkT = nc.dram_tensor("wkT", [D, 256], BF, kind="ExternalInput")
    wvT = nc.dram_tensor("wvT", [D, 256], BF, kind="ExternalInput")
    woT = nc.dram_tensor("woT", [256, D], BF, kind="ExternalInput")
    bqc_d = nc.dram_tensor("bqc", [P, 2], F32, kind="ExternalInput")
    bkc_d = nc.dram_tensor("bkc", [P, 2], F32, kind="ExternalInput")
    bv_d = nc.dram_tensor("bv", [1, 256], BF, kind="ExternalInput")
    if causal:
        cm_d = nc.dram_tensor("cm", [P, 2, P], BF, kind="ExternalInput")
    outT = nc.dram_tensor("outT", [D, S], BF, kind="ExternalOutput")

    with tile.TileContext(nc) as tc, \
         tc.tile_pool(name="const", bufs=1) as const, \
         tc.tile_pool(name="big", bufs=1) as big, \
         tc.tile_pool(name="oio", bufs=3) as oio, \
         tc.tile_pool(name="ex", bufs=4) as ex, \
         tc.tile_pool(name="u", bufs=4) as up, \
         tc.tile_pool(name="sm", bufs=2) as sm, \
         tc.tile_pool(name="ps_a", bufs=2, space="PSUM") as ps_a, \
         tc.tile_pool(name="ps_c", bufs=2, space="PSUM") as ps_c, \
         tc.tile_pool(name="ps_w", bufs=2, space="PSUM") as ps_w:

        # ---------- constants / inputs ----------
        ones = const.tile([1, P], BF)
        nc.gpsimd.memset(ones[:], 1.0)
        bqc_sb = const.tile([P, 2], F32)
        nc.scalar.dma_start(bqc_sb[:], bqc_d[:])
        bkc_sb = const.tile([P, 2], F32)
        nc.scalar.dma_start(bkc_sb[:], bkc_d[:])
        bv_sb = const.tile([1, 256], BF)
        nc.scalar.dma_start(bv_sb[:], bv_d[:])
        if causal:
            cm_sb = const.tile([P, 2, P], BF)
            nc.scalar.dma_start(cm_sb[:], cm_d[:])

        wk_sb = big.tile([P, 8, 256], BF)
        wv_sb = big.tile([P, 8, 256], BF)
        wq_sb = big.tile([P, 8, 256], BF)
        wo_sb = big.tile([P, 2, D], BF)
        xt_sb = big.tile([P, 8, S], BF)
        kt_sb = big.tile([P, 2, S], BF)
        qt_sb = big.tile([P, 2, S], BF)
        v_sb = big.tile([P, 16, 4, 65], BF)
        ctx_sb = big.tile([P, 2, S], BF)
        nc.gpsimd.memset(v_sb[:, :, :, 64:65], 1.0)

        wkr = wkT.rearrange("(o p) f -> p o f", p=P)
        wvr = wvT.rearrange("(o p) f -> p o f", p=P)
        wqr = wqT.rearrange("(o p) f -> p o f", p=P)
        wor = woT.rearrange("(o p) f -> p o f", p=P)
        xr = xT.rearrange("(o p) t -> p o t", p=P)
        outr = outT.rearrange("(o p) t -> p o t", p=P)

        # Input DMAs: the critical path is wk + x chunk 0 (feeds the
        # first K matmuls).  Only sync/scalar issue DMAs (gpsimd DMAs go
        # through the slow software DGE).  scalar: consts then x chunk 0;
        # sync: weights then the bulk of x, so the bulk never competes
        # with the startup-critical bytes.
        nc.sync.dma_start(wk_sb[:], wkr[:])
        nc.sync.dma_start(wv_sb[:], wvr[:])
        nc.sync.dma_start(wq_sb[:], wqr[:])
        for kt in range(8):
            nc.scalar.dma_start(xt_sb[:, kt, 0:512], xr[:, kt, 0:512])
        for kt in range(8):
            nc.sync.dma_start(xt_sb[:, kt, 512:2048], xr[:, kt, 512:2048])
        nc.sync.dma_start(wo_sb[:], wor[:])

        def proj_chunk(tc_i):
            t0 = 512 * tc_i
            # K^T and Q^T: out[feat, tok], feature block fb == head pair;
            # bias is a per-partition scalar add fused into the copy.
            for w_sb, b_sb, sink in ((wk_sb, bkc_sb, kt_sb),
                                     (wq_sb, bqc_sb, qt_sb)):
                for fb in range(2):
                    pt = ps_w.tile([P, 512], F32, tag="psw")
                    for kt in range(8):
                        nc.tensor.matmul(
                            pt[:], w_sb[:, kt, 128 * fb:128 * fb + 128],
                            xt_sb[:, kt, t0:t0 + 512],
                            start=(kt == 0), stop=(kt == 7))
                    nc.vector.tensor_scalar_add(
                        sink[:, fb, t0:t0 + 512], pt[:], b_sb[:, fb:fb + 1])
            # V: out[tok, feat] per 128-token block (65th col pre-set to 1)
            for tb in range(4):
                jb = 4 * tc_i + tb
                pt = ps_w.tile([P, 512], F32, tag="psw")
                for kt in range(8):
                    nc.tensor.matmul(
                        pt[:, 0:256],
                        xt_sb[:, kt, t0 + 128 * tb:t0 + 128 * tb + 128],
                        wv_sb[:, kt, :], start=(kt == 0), stop=False)
                nc.tensor.matmul(
                    pt[:, 0:256], ones[0:1, 0:P], bv_sb[0:1, :],
                    start=False, stop=True)
                nc.vector.tensor_copy(
                    v_sb[:, jb, :, 0:64],
                    pt[:, 0:256].rearrange("p (h d) -> p h d", h=4))

        def attn_chunk(tc_i):
            """Scores+exp+ctx for both head pairs.  One flat software
            pipeline over blocks (pair-major): the ctx matmuls of block b
            issue after the scores of block b+1, across the pair boundary
            too, so the PE never waits on the scalar engine's exp."""
            t0 = 512 * tc_i
            jn = 4 * tc_i + 4 if causal else 16
            ctx = {}
            prev = None
            us = []

            def emit_ctx(pair, pj, pet, pqo, pwid):
                for hh in range(2):
                    nc.tensor.matmul(
                        ctx[pair][hh][0:65, pqo:pqo + pwid],
                        v_sb[:, pj, 2 * pair + hh, :], pet[:, hh, 0:pwid],
                        start=(pj == 0), stop=(pj == jn - 1))
                if pj == jn - 1:
                    for hh in range(2):
                        u = up.tile([65, 512], F32, tag="u")
                        nc.vector.tensor_copy(u[:], ctx[pair][hh][0:65, :])
                        us.append(u)

            for pair in range(2):
                c0 = ps_c.tile([P, 512], F32, tag="ctx")
                c1 = ps_c.tile([P, 512], F32, tag="ctx")
                ctx[pair] = (c0, c1)
                for j in range(jn):
                    o_ = j - 4 * tc_i if causal else -1
                    qo = 0 if o_ < 0 else 128 * o_
                    wid = 512 - qo
                    sc = ps_a.tile([P, 1024], F32, tag="psa")
                    for hh in range(2):
                        nc.tensor.matmul(
                            sc[:, 512 * hh:512 * hh + wid],
                            kt_sb[64 * hh:64 * hh + 64, pair,
                                  128 * j:128 * j + 128],
                            qt_sb[64 * hh:64 * hh + 64, pair,
                                  t0 + qo:t0 + qo + wid],
                            start=True, stop=True)
                    et = ex.tile([P, 2, 512], BF, tag="exp")
                    if wid == 512:
                        nc.scalar.activation(et[:, :, :], sc[:, :], fexp)
                    else:
                        nc.scalar.activation(
                            et[:, :, 0:wid],
                            sc[:].rearrange("p (s n) -> p s n", s=2)
                            [:, :, 0:wid], fexp)
                    if o_ >= 0:
                        nc.vector.tensor_tensor(
                            et[:, :, 0:P], et[:, :, 0:P], cm_sb[:],
                            mybir.AluOpType.mult)
                    if prev is not None:
                        emit_ctx(*prev)
                    prev = (pair, j, et, qo, wid)
            emit_ctx(*prev)
            return us

        def norm_chunk(tc_i, us):
            """recip = exp(-ln(denominator)); even head lands on partitions
            0:64 of ctx_sb, odd heads are collected in a [64,2,512] tile
            and moved to partitions 64:128 with ONE SBUF->SBUF DMA
            (issued from the scalar queue: hardware DGE)."""
            t0 = 512 * tc_i
            ctmp = sm.tile([64, 2, 512], BF, tag="ctmp")
            for pair in range(2):
                for hh in range(2):
                    u = us[2 * pair + hh]
                    lnd = sm.tile([1, 512], F32, tag="lnd")
                    nc.scalar.activation(lnd[:], u[64:65, 0:512], fln)
                    rcp = sm.tile([1, 512], BF, tag="rcp")
                    nc.scalar.activation(rcp[:], lnd[:], fexp, scale=-1.0)
                    rep_ps = ps_w.tile([P, 512], F32, tag="psw")
                    nc.tensor.matmul(rep_ps[0:64, :], ones[0:1, 0:64],
                                     rcp[0:1, :], start=True, stop=True)
                    rep = sm.tile([64, 512], F32, tag="rep")
                    nc.vector.tensor_copy(rep[:], rep_ps[0:64, :])
                    if hh == 0:
                        nc.vector.tensor_tensor(
                            ctx_sb[0:64, pair, t0:t0 + 512], u[0:64, :],
                            rep[:], mybir.AluOpType.mult)
                    else:
                        nc.vector.tensor_tensor(
                            ctmp[:, pair, :], u[0:64, :], rep[:],
                            mybir.AluOpType.mult)
            nc.scalar.dma_start(ctx_sb[64:128, 0:2, t0:t0 + 512], ctmp[:])

        def outproj_chunk(tc_i):
            t0 = 512 * tc_i
            for m in range(8):
                pt = ps_w.tile([P, 512], F32, tag="psw")
                for kt in range(2):
                    nc.tensor.matmul(
                        pt[:], wo_sb[:, kt, 128 * m:128 * m + 128],
                        ctx_sb[:, kt, t0:t0 + 512],
                        start=(kt == 0), stop=(kt == 1))
                t = oio.tile([P, 512], BF, tag="oio")
                nc.vector.tensor_copy(t[:], pt[:])
                nc.sync.dma_start(outr[:, m, t0:t0 + 512], t[:])

        if causal:
            proj_chunk(0)
            for tc_i in range(4):
                us = attn_chunk(tc_i)
                if tc_i < 3:
                    proj_chunk(tc_i + 1)
                norm_chunk(tc_i, us)
                outproj_chunk(tc_i)
        else:
            for tc_i in range(4):
                proj_chunk(tc_i)
            for tc_i in range(4):
                us = attn_chunk(tc_i)
                norm_chunk(tc_i, us)
                outproj_chunk(tc_i)

    nc.compile()
    return nc


_CACHE = {}


def _get_nc(causal: bool):
    if causal not in _CACHE:
        _CACHE[causal] = _emit(causal)
    return _CACHE[causal]


def kernel(**inputs):
    x = np.asarray(inputs["x"], dtype=np.float32)
    Wq = np.asarray(inputs["Wq"], dtype=np.float32)
    bq = np.asarray(inputs["bq"], dtype=np.float32)
    Wk = np.asarray(inputs["Wk"], dtype=np.float32)
    bk = np.asarray(inputs["bk"], dtype=np.float32)
    Wv = np.asarray(inputs["Wv"], dtype=np.float32)
    bv = np.asarray(inputs["bv"], dtype=np.float32)
    Wo = np.asarray(inputs["Wo"], dtype=np.float32)
    bo = np.asarray(inputs["bo"], dtype=np.float32)
    causal = bool(int(np.asarray(inputs["enable_causal"])))

    scale = np.float32(1.0 / np.sqrt(HD))
    xTb = [np.ascontiguousarray(x[b].T).astype(NPBF) for b in range(B)]
    cm = np.ascontiguousarray(np.broadcast_to(
        (np.arange(P)[:, None] <= np.arange(P)[None, :])
        .astype(np.float32)[:, None, :], (P, 2, P))).astype(NPBF)

    nc = _get_nc(causal)
    in_maps = []
    for c in range(NC):
        b, g = divmod(c, 4)
        f0 = 256 * g
        m = {"xT": xTb[b],
             "wqT": np.ascontiguousarray(Wq[f0:f0 + 256, :].T).astype(NPBF),
             "wkT": np.ascontiguousarray(
                 (Wk[f0:f0 + 256, :] * scale).T).astype(NPBF),
             "wvT": np.ascontiguousarray(Wv[f0:f0 + 256, :].T).astype(NPBF),
             "woT": np.ascontiguousarray(Wo[:, f0:f0 + 256].T).astype(NPBF),
             "bqc": np.ascontiguousarray(
                 bq[f0:f0 + 256].reshape(2, P).T).astype(np.float32),
             "bkc": np.ascontiguousarray(
                 (bk[f0:f0 + 256] * scale).reshape(2, P).T).astype(np.float32),
             "bv": bv[f0:f0 + 256].reshape(1, 256).astype(NPBF)}
        if causal:
            m["cm"] = cm
        in_maps.append(m)

    global LAST_RESULT
    res = run_bass_kernel_spmd(nc, in_maps, list(range(NC)), trace=TRACE)
    LAST_RESULT = res
    out = np.zeros((B, S, D), dtype=np.float32)
    for c in range(NC):
        b = c // 4
        out[b] += np.asarray(res.results[c]["outT"], dtype=np.float32).T
    out += bo[None, None, :]
    return out
